# revision 55
# baseline (speedup 1.0000x reference)
"""Trainium2 Bass kernel for CustomAttention (qkv -> per-head LN on q,k -> SDPA -> proj).

Sharding: 8 cores = 2 batches x 4 head-groups (3 heads each).
Per core: qkv projection for its heads from x[b], full attention per head
(scores computed transposed so no probability-matrix transpose is needed,
softmax denominator folded into the PV matmul as a ones column on V),
then a partial output projection over its 192 channels. Host sums the 4
partials per batch and adds proj_b.

Schedule notes:
 - Phase B is software-pipelined 4 slots deep so each engine's in-order
   queue only ever sees work whose inputs are already computed. LN is a
   merged q|k chain (6 virtual heads); the elementwise apply runs in bf16
   for DVE 2x throughput.
 - Attention is one continuous stream across all (i-block, head) pairs:
   score matmuls alternate PE quadrants (tile_position row 0/64) so
   adjacent score matmuls execute concurrently, PV matmuls pop from a
   global skewed queue (so the scalar engine's exp stream never stalls at
   head boundaries), and the output projection is interleaved one unit
   per score tile during the following i-block.
 - exp() widened to 1536 columns to amortize ACT fixed overhead; scalar
   does nothing else in phase C.
"""

import os
import sys
from functools import lru_cache

import numpy as np

for _p in ("/opt/trn_rl_repo", os.path.expanduser("~/.axon_site/_ro/trn_rl_repo")):
    if os.path.isdir(_p) and _p not in sys.path:
        sys.path.insert(0, _p)

import concourse.bass as bass
import concourse.mybir as mybir
from concourse import bacc
import concourse.tile as tile
from concourse.masks import make_identity

F32 = mybir.dt.float32
F32R = mybir.dt.float32r
BF16 = mybir.dt.bfloat16
ALU = mybir.AluOpType
ACTF = mybir.ActivationFunctionType
AXL = mybir.AxisListType

H = 3          # heads per core
D = 64         # head dim
C = 768        # model dim
J = 3 * H * D  # qkv rows per core = 576
G = 2 * H      # merged LN virtual heads (q0..2, k0..2)
EPS = 1e-5
SCALE = D ** -0.5

SKEW_CHUNKS = 16  # PV matmuls lag the score/exp pipeline by this many j-chunks


def r32(ap):
    return ap.bitcast(F32R)


def build_nc(N=4096):
    """One-core program; all 8 cores run it SPMD with different input data."""
    NB = N // 128          # j-chunks / n-blocks
    IB = N // 512          # i-blocks

    nc = bacc.Bacc("TRN2", target_bir_lowering=False, debug=False)
    x_t = nc.declare_dram_parameter("x_t", [C, N], BF16, isOutput=False)
    wqkv_t = nc.declare_dram_parameter("wqkv_t", [C, J], BF16, isOutput=False)
    projw_t = nc.declare_dram_parameter("projw_t", [H * D, C], F32, isOutput=False)
    # per-partition LN affine columns, rows = d duplicated over both halves:
    # col 0 = gamma_q*scale, 1 = beta_q*scale, 2 = gamma_k, 3 = beta_k
    gbc = nc.declare_dram_parameter("gbc", [128, 4], F32, isOutput=False)
    out_p = nc.declare_dram_parameter("out_p", [N, C], F32, isOutput=True)

    with tile.TileContext(nc) as tc:
        with (
            tc.tile_pool(name="persist", bufs=1) as persist,
            tc.tile_pool(name="weights", bufs=1) as weights,
        ):
            # ---- persistent SBUF tensors ----
            # qT duplicated across both partition halves: rows 0:64 == 64:128
            qT = persist.tile([128, H, N], BF16, tag="qT")
            # kT stacked: rows 0:64 = j in [0,N/2), rows 64:128 = j in [N/2,N)
            kT = persist.tile([128, H, N // 2], BF16, tag="kT")
            # V augmented with a ones column (index 64) per j-chunk
            vA = persist.tile([128, H, NB, 65], BF16, tag="vA")
            # attention output, channel-major: ao1 rows = h0,h1; ao2 rows = h2
            ao1 = persist.tile([128, N], F32R, tag="ao1")
            ao2 = persist.tile([64, N], F32R, tag="ao2")

            ident = persist.tile([128, 128], F32, tag="ident")
            make_identity(nc, ident[:])
            identb = persist.tile([128, 128], BF16, tag="identb")
            nc.vector.tensor_copy(identb[:], ident[:])
            nc.vector.memset(vA[:, :, :, 64:65], 1.0)

            # split per contraction chunk so the first qkv matmul can start
            # as soon as the first slice lands; remaining chunks are issued
            # after the first x tile's DMA (see phase B loop)
            wq = weights.tile([128, 6, J], BF16, tag="wqkv")
            wq_src = wqkv_t.rearrange("(ck p) j -> p ck j", p=128)
            nc.sync.dma_start(wq[:, 0, :], wq_src[:, 0, :])
            gbct = weights.tile([128, 4], F32, tag="gbc")
            epst = weights.tile([128, 1], F32, tag="epst")
            nc.vector.memset(epst[:], EPS)
            # proj weights are only needed in phase C; load them behind the
            # first x tiles
            pw128 = weights.tile([128, C], F32R, tag="pw128")
            pw64 = weights.tile([64, C], F32R, tag="pw64")

            # ================= Phase B: qkv + LN + transpose =================
            # software pipeline, slot s handles: qkv(s), stats(s-1), apply(s-2),
            # transpose+copies(s-3)
            with (
                tc.tile_pool(name="pB", bufs=3) as pB,
                tc.tile_pool(name="pBs", bufs=4) as pBs,
                tc.tile_pool(name="psQ", bufs=3, space="PSUM") as psQ,
                tc.tile_pool(name="psT", bufs=2, space="PSUM") as psT,
            ):
                st = {}   # per-nb dict of live tiles

                def emit_qkv(nb):
                    xt = pB.tile([128, 6, 128], BF16, tag="xt")
                    nc.sync.dma_start(
                        xt[:],
                        x_t.rearrange("(ck p) n -> p ck n", p=128)[
                            :, :, nb * 128 : (nb + 1) * 128
                        ],
                    )
                    if nb == 0:
                        # remaining weight slices land while the first x tile
                        # is being consumed
                        for ck in range(1, 6):
                            nc.sync.dma_start(wq[:, ck, :], wq_src[:, ck, :])
                        nc.sync.dma_start(gbct[:], gbc[:, :])
                    # q|k at cols 0:384 (bank 0), v at 512:704 (bank 1)
                    ps = psQ.tile([128, 1024], F32, tag="qkvps")
                    for off, woff, w in ((0, 0, 384), (512, 384, 192)):
                        for ck in range(6):
                            nc.tensor.matmul(
                                ps[:, off : off + w],
                                xt[:, ck, :],
                                wq[:, ck, woff : woff + w],
                                start=(ck == 0),
                                stop=(ck == 5),
                            )
                    st[nb] = {"ps": ps}

                def emit_stats(nb):
                    t = st[nb]
                    ps = t.pop("ps")
                    qkvS = pBs.tile([128, G, D], BF16, tag="qkvS")
                    nc.scalar.copy(
                        qkvS[:], ps[:, 0 : G * D].rearrange("p (g d) -> p g d", d=D)
                    )
                    nc.scalar.copy(
                        vA[:, :, nb, 0:64],
                        ps[:, 512 : 512 + H * D].rearrange("p (h d) -> p h d", d=D),
                    )
                    s1 = pBs.tile([128, G], F32, tag="s1")
                    nc.vector.tensor_reduce(s1[:], qkvS[:], AXL.X, ALU.add)
                    mu = pBs.tile([128, G], F32, tag="mu")
                    nc.vector.tensor_scalar_mul(mu[:], s1[:], 1.0 / D)
                    sq = pBs.tile([128, G, D], BF16, tag="sq")
                    nc.vector.tensor_mul(sq[:], qkvS[:], qkvS[:])
                    s2 = pBs.tile([128, G], F32, tag="s2")
                    nc.vector.tensor_reduce(s2[:], sq[:], AXL.X, ALU.add)
                    musq = pBs.tile([128, G], F32, tag="musq")
                    nc.vector.tensor_mul(musq[:], mu[:], mu[:])
                    var = pBs.tile([128, G], F32, tag="var")
                    nc.vector.scalar_tensor_tensor(
                        var[:], s2[:], 1.0 / D, musq[:], ALU.mult, ALU.subtract
                    )
                    std = pBs.tile([128, G], F32, tag="std")
                    nc.scalar.activation(std[:], var[:], ACTF.Sqrt, bias=epst[:])
                    rstd = pBs.tile([128, G], F32, tag="rstd")
                    nc.vector.reciprocal_approx_fast(rstd[:], std[:])
                    t.update(qkvS=qkvS, mu=mu, rstd=rstd)

                def emit_apply(nb):
                    t = st[nb]
                    qkvS, mu, rstd = t.pop("qkvS"), t.pop("mu"), t.pop("rstd")
                    # cs = (x - mu) * rstd on gpsimd; gamma/beta are applied
                    # later, inside the post-transpose per-partition copies
                    cst = pBs.tile([128, G, D], BF16, tag="cst")
                    nc.gpsimd.tensor_sub(
                        cst[:], qkvS[:], mu[:, :, None].broadcast_to([128, G, D])
                    )
                    cs = pBs.tile([128, G, D], BF16, tag="cs")
                    nc.gpsimd.tensor_mul(
                        cs[:], cst[:], rstd[:, :, None].broadcast_to([128, G, D])
                    )
                    t["cs"] = cs

                def emit_transp(nb):
                    t = st[nb]
                    cs = t.pop("cs")
                    pst = psT.tile([128, G, 128], BF16, tag="pst")
                    jh = nb // (NB // 2)
                    for g in range(H):
                        # q slabs duplicated over both 64-row halves via two
                        # transposes of the same stationary
                        for half in range(2):
                            nc.tensor.transpose(
                                pst[64 * half : 64 * half + 64, g, :],
                                cs[:, g, :],
                                identb[:],
                                tile_position=(0, 64 * half),
                            )
                    for g in range(H, G):
                        # k slabs only need the jh half
                        nc.tensor.transpose(
                            pst[64 * jh : 64 * jh + 64, g, :],
                            cs[:, g, :],
                            identb[:],
                            tile_position=(0, 64 * jh),
                        )
                    blk = slice(nb * 128, (nb + 1) * 128)
                    # copies apply gamma/beta per partition (= per d after
                    # the transpose): out = in * gamma_col + beta_col
                    nc.scalar.activation(
                        qT[:, :, blk], pst[:, 0:H, :], ACTF.Identity,
                        bias=gbct[:, 1:2], scale=gbct[:, 0:1],
                    )
                    cb = nb % (NB // 2)
                    rows = slice(64 * jh, 64 * jh + 64)
                    nc.scalar.activation(
                        kT[rows, :, cb * 128 : (cb + 1) * 128], pst[rows, H:G, :],
                        ACTF.Identity,
                        bias=gbct[rows, 3:4], scale=gbct[rows, 2:3],
                    )
                    del st[nb]

                for s in range(NB + 3):
                    if s == 3:
                        nc.sync.dma_start(pw128[:], projw_t[0:128, :].bitcast(F32R))
                        nc.sync.dma_start(pw64[:], projw_t[128:192, :].bitcast(F32R))
                    if s >= 2 and s - 2 < NB:
                        emit_apply(s - 2)
                    if s >= 3:
                        emit_transp(s - 3)
                    if s < NB:
                        emit_qkv(s)
                    if s >= 1 and s - 1 < NB:
                        emit_stats(s - 1)

            # ================= Phase C: attention + proj =================
            NT = (NB + 2) // 3          # score/exp tiles per (ib, h)
            with (
                tc.tile_pool(name="pt", bufs=9) as ptp,
                tc.tile_pool(name="pCs", bufs=6) as pCs,
                tc.tile_pool(name="pD", bufs=3) as pD,
                tc.tile_pool(name="psS", bufs=2, space="PSUM") as psS,
                tc.tile_pool(name="psOD", bufs=2, space="PSUM") as psOD,
            ):
                # chunk order alternates PE quadrants so adjacent score
                # matmuls run concurrently: 0,16,1,17,2,18,...
                corder = []
                for i in range(NB // 2):
                    corder.append(i)
                    corder.append(i + NB // 2)

                pvq = []      # pending PV closures, global FIFO across heads
                side = []     # pending (ib, proj-unit) from the previous i-block
                ao_done = set()   # i-blocks whose h2 normalize has been emitted

                def pop_pv():
                    pvq.pop(0)()

                def pop_side():
                    if side and side[0][0] in ao_done:
                        side.pop(0)[1]()

                def normalize(ib, h, pso):
                    isl = slice(ib * 512, (ib + 1) * 512)
                    rden_f = pCs.tile([1, 512], F32, tag="rden_f")
                    nc.vector.tensor_copy(rden_f[:], pso[64:65, :])
                    rden = pCs.tile([1, 512], F32, tag="rden")
                    nc.vector.reciprocal_approx_fast(rden[:], rden_f[:])
                    rb = pCs.tile([64, 512], F32, tag="rb")
                    nc.gpsimd.partition_broadcast(rb[:], rden[:])
                    if h == 0:
                        nc.vector.tensor_mul(ao1[0:64, isl], pso[0:64, :], rb[:])
                    elif h == 2:
                        nc.vector.tensor_mul(ao2[0:64, isl], pso[0:64, :], rb[:])
                    else:
                        stg = pCs.tile([64, 512], F32R, tag="stg")
                        nc.vector.tensor_mul(stg[:], pso[0:64, :], rb[:])
                        nc.sync.dma_start(ao1[64:128, isl], stg[:])
                    if h == 2:
                        ao_done.add(ib)

                def make_pv(pso, h, jc, idx, ib):
                    def run():
                        nc.tensor.matmul(
                            pso,
                            vA[:, h, jc, :],
                            pvq_pt.pop((ib, h, idx)),
                            start=(idx == 0),
                            stop=(idx == NB - 1),
                        )
                        if idx == NB - 1:
                            normalize(ib, h, pso)
                    return run

                pvq_pt = {}

                def make_proj(ib):
                    units = []
                    for nb in range(ib * 4, ib * 4 + 4):
                        blk = slice(nb * 128, (nb + 1) * 128)
                        stage = [None]

                        def u1(blk=blk, stage=stage):
                            stage[0] = pD.tile([128, C], F32, tag="stage", name="stage")
                            pd_t = psOD.tile([128, 512], F32, tag="psod", name="pd")
                            nc.tensor.matmul(
                                pd_t[:, 0:512], r32(ao1[:, blk]),
                                r32(pw128[:, 0:512]), start=True, stop=False,
                            )
                            nc.tensor.matmul(
                                pd_t[:, 0:512], r32(ao2[0:64, blk]),
                                r32(pw64[0:64, 0:512]), start=False, stop=True,
                            )
                            nc.vector.tensor_copy(stage[0][:, 0:512], pd_t[:, 0:512])

                        def u2(blk=blk, stage=stage):
                            pd_t = psOD.tile([128, 512], F32, tag="psod", name="pd")
                            nc.tensor.matmul(
                                pd_t[:, 0:256], r32(ao1[:, blk]),
                                r32(pw128[:, 512:768]), start=True, stop=False,
                            )
                            nc.tensor.matmul(
                                pd_t[:, 0:256], r32(ao2[0:64, blk]),
                                r32(pw64[0:64, 512:768]), start=False, stop=True,
                            )
                            nc.vector.tensor_copy(stage[0][:, 512:768], pd_t[:, 0:256])
                            nc.sync.dma_start(out_p[blk, :], stage[0][:])

                        units.append((ib, u1))
                        units.append((ib, u2))
                    return units

                for ib in range(IB):
                    isl = slice(ib * 512, (ib + 1) * 512)
                    for h in range(H):
                        pso_t = psOD.tile([128, 512], F32, tag="psod", name="pso")
                        pso = pso_t[0:65, :]
                        for t in range(NT):
                            nch = min(3, NB - 3 * t)
                            W = 512 * nch
                            ps = psS.tile([128, 1536], F32, tag="st")
                            for s in range(nch):
                                jc = corder[3 * t + s]
                                p0 = 0 if jc < NB // 2 else 64
                                jf = (jc % (NB // 2)) * 128
                                nc.tensor.matmul(
                                    ps[:, 512 * s : 512 * s + 512],
                                    kT[p0 : p0 + 64, h, jf : jf + 128],
                                    qT[p0 : p0 + 64, h, isl],
                                    start=True,
                                    stop=True,
                                    tile_position=(p0, 0),
                                )
                                # keep PV/proj matmuls out of the middle of a
                                # score pair (adjacent scores alternate PE
                                # quadrants and execute concurrently), and
                                # never pop during a head's first tile so the
                                # exp stream restarts without delay; bounded
                                # pops avoid bursts that would delay s2
                                if t == 0 or t == NT - 1:
                                    pass
                                elif s == 0 and len(pvq) > SKEW_CHUNKS + 2:
                                    pop_pv()
                                elif s == 1:
                                    if len(pvq) > SKEW_CHUNKS:
                                        pop_pv()
                                    if len(pvq) > SKEW_CHUNKS + 1:
                                        pop_pv()
                                    if t >= 4:
                                        pop_side()
                                elif s == 2 and len(pvq) > SKEW_CHUNKS:
                                    pop_pv()
                            pt = ptp.tile([128, 1536], BF16, tag="pt")
                            nc.scalar.activation(pt[:, 0:W], ps[:, 0:W], ACTF.Exp)
                            for s in range(nch):
                                idx = 3 * t + s
                                pvq_pt[(ib, h, idx)] = pt[:, 512 * s : 512 * s + 512]
                                pvq.append(make_pv(pso, h, corder[idx], idx, ib))
                    # queue this i-block's projection for the next i-block
                    side.extend(make_proj(ib))
                    if ib == IB - 1:
                        while pvq:
                            pop_pv()
                        while side:
                            side.pop(0)[1]()

    nc.compile()
    return nc


@lru_cache(maxsize=2)
def _built(N):
    nc = build_nc(N)
    return nc


def _prep_inputs(x, qkv_w, q_gamma, q_beta, k_gamma, k_beta, proj_w):
    x = np.asarray(x, np.float32)
    qkv_w = np.asarray(qkv_w, np.float32)
    proj_w = np.asarray(proj_w, np.float32)
    B = x.shape[0]
    import ml_dtypes
    xts = [np.ascontiguousarray(x[b].T).astype(ml_dtypes.bfloat16) for b in range(B)]
    gb2 = np.stack(
        [
            np.tile(np.asarray(q_gamma, np.float32) * SCALE, 2),
            np.tile(np.asarray(q_beta, np.float32) * SCALE, 2),
            np.tile(np.asarray(k_gamma, np.float32), 2),
            np.tile(np.asarray(k_beta, np.float32), 2),
        ],
        axis=1,
    )  # [128, 4]
    gbs = []
    wqs = []
    pws = []
    for g in range(4):
        r = slice(192 * g, 192 * (g + 1))
        wq_rows = np.concatenate(
            [qkv_w[r], qkv_w[768:1536][r], qkv_w[1536:2304][r]], axis=0
        )
        wqs.append(np.ascontiguousarray(wq_rows.T).astype(ml_dtypes.bfloat16))
        pws.append(np.ascontiguousarray(proj_w[:, r].T))
        gbs.append(gb2)
    in_maps = []
    for core in range(8):
        b, g = core // 4, core % 4
        in_maps.append(
            {"x_t": xts[b], "wqkv_t": wqs[g], "projw_t": pws[g], "gbc": gbs[g]}
        )
    return in_maps


def run_cores(in_maps, N, trace=False):
    from concourse.bass_utils import run_bass_kernel_spmd

    nc = _built(N)
    res = run_bass_kernel_spmd(nc, in_maps, list(range(8)), trace=trace)
    return res


def kernel(x, qkv_w, q_gamma, q_beta, k_gamma, k_beta, proj_w, proj_b):
    x = np.asarray(x, np.float32)
    N = x.shape[1]
    in_maps = _prep_inputs(x, qkv_w, q_gamma, q_beta, k_gamma, k_beta, proj_w)
    res = run_cores(in_maps, N)
    parts = [np.asarray(r["out_p"], np.float32) for r in res.results]
    out0 = parts[0] + parts[1] + parts[2] + parts[3]
    out1 = parts[4] + parts[5] + parts[6] + parts[7]
    out = np.stack([out0, out1]) + np.asarray(proj_b, np.float32)
    return out.astype(np.float32)


# revision 56
# speedup vs baseline: 1.0106x; 1.0106x over previous
"""Trainium2 Bass kernel for CustomAttention (qkv -> per-head LN on q,k -> SDPA -> proj).

Sharding: 8 cores = 2 batches x 4 head-groups (3 heads each).
Per core: qkv projection for its heads from x[b], full attention per head
(scores computed transposed so no probability-matrix transpose is needed,
softmax denominator folded into the PV matmul as a ones column on V),
then a partial output projection over its 192 channels. Host sums the 4
partials per batch and adds proj_b.

Schedule notes:
 - Phase B is software-pipelined 4 slots deep so each engine's in-order
   queue only ever sees work whose inputs are already computed. LN is a
   merged q|k chain (6 virtual heads); the elementwise apply runs in bf16
   for DVE 2x throughput.
 - Attention is one continuous stream across all (i-block, head) pairs:
   score matmuls alternate PE quadrants (tile_position row 0/64) so
   adjacent score matmuls execute concurrently, PV matmuls pop from a
   global skewed queue (so the scalar engine's exp stream never stalls at
   head boundaries), and the output projection is interleaved one unit
   per score tile during the following i-block.
 - exp() widened to 1536 columns to amortize ACT fixed overhead; scalar
   does nothing else in phase C.
"""

import os
import sys
from functools import lru_cache

import numpy as np

for _p in ("/opt/trn_rl_repo", os.path.expanduser("~/.axon_site/_ro/trn_rl_repo")):
    if os.path.isdir(_p) and _p not in sys.path:
        sys.path.insert(0, _p)

import concourse.bass as bass
import concourse.mybir as mybir
from concourse import bacc
import concourse.tile as tile
from concourse.masks import make_identity

F32 = mybir.dt.float32
F32R = mybir.dt.float32r
BF16 = mybir.dt.bfloat16
ALU = mybir.AluOpType
ACTF = mybir.ActivationFunctionType
AXL = mybir.AxisListType

H = 3          # heads per core
D = 64         # head dim
C = 768        # model dim
J = 3 * H * D  # qkv rows per core = 576
G = 2 * H      # merged LN virtual heads (q0..2, k0..2)
EPS = 1e-5
SCALE = D ** -0.5

SKEW_CHUNKS = 16  # PV matmuls lag the score/exp pipeline by this many j-chunks


def r32(ap):
    return ap.bitcast(F32R)


def build_nc(N=4096):
    """One-core program; all 8 cores run it SPMD with different input data."""
    NB = N // 128          # j-chunks / n-blocks
    IB = N // 512          # i-blocks

    nc = bacc.Bacc("TRN2", target_bir_lowering=False, debug=False)
    x_t = nc.declare_dram_parameter("x_t", [C, N], BF16, isOutput=False)
    wqkv_t = nc.declare_dram_parameter("wqkv_t", [C, J], BF16, isOutput=False)
    projw_t = nc.declare_dram_parameter("projw_t", [H * D, C], F32, isOutput=False)
    # per-partition LN affine columns, rows = d duplicated over both halves:
    # col 0 = gamma_q*scale, 1 = beta_q*scale, 2 = gamma_k, 3 = beta_k
    gbc = nc.declare_dram_parameter("gbc", [128, 4], F32, isOutput=False)
    out_p = nc.declare_dram_parameter("out_p", [N, C], F32, isOutput=True)

    with tile.TileContext(nc) as tc:
        with (
            tc.tile_pool(name="persist", bufs=1) as persist,
            tc.tile_pool(name="weights", bufs=1) as weights,
        ):
            # ---- persistent SBUF tensors ----
            # qT duplicated across both partition halves: rows 0:64 == 64:128
            qT = persist.tile([128, H, N], BF16, tag="qT")
            # kT stacked: rows 0:64 = j in [0,N/2), rows 64:128 = j in [N/2,N)
            kT = persist.tile([128, H, N // 2], BF16, tag="kT")
            # V augmented with a ones column (index 64) per j-chunk
            vA = persist.tile([128, H, NB, 65], BF16, tag="vA")
            # attention output, channel-major: ao1 rows = h0,h1; ao2 rows = h2
            ao1 = persist.tile([128, N], F32R, tag="ao1")
            ao2 = persist.tile([64, N], F32R, tag="ao2")

            ident = persist.tile([128, 128], F32, tag="ident")
            make_identity(nc, ident[:])
            identb = persist.tile([128, 128], BF16, tag="identb")
            nc.vector.tensor_copy(identb[:], ident[:])
            nc.vector.memset(vA[:, :, :, 64:65], 1.0)

            # split per contraction chunk so the first qkv matmul can start
            # as soon as the first slice lands; remaining chunks are issued
            # after the first x tile's DMA (see phase B loop)
            wq = weights.tile([128, 6, J], BF16, tag="wqkv")
            wq_src = wqkv_t.rearrange("(ck p) j -> p ck j", p=128)
            nc.sync.dma_start(wq[:, 0, :], wq_src[:, 0, :])
            gbct = weights.tile([128, 4], F32, tag="gbc")
            epst = weights.tile([128, 1], F32, tag="epst")
            nc.vector.memset(epst[:], EPS)
            # proj weights are only needed in phase C; load them behind the
            # first x tiles
            pw128 = weights.tile([128, C], F32R, tag="pw128")
            pw64 = weights.tile([64, C], F32R, tag="pw64")

            # ================= Phase B: qkv + LN + transpose =================
            # software pipeline, slot s handles: qkv(s), stats(s-1), apply(s-2),
            # transpose+copies(s-3)
            with (
                tc.tile_pool(name="pB", bufs=3) as pB,
                tc.tile_pool(name="pBs", bufs=4) as pBs,
                tc.tile_pool(name="psQ", bufs=3, space="PSUM") as psQ,
                tc.tile_pool(name="psT", bufs=2, space="PSUM") as psT,
            ):
                st = {}   # per-nb dict of live tiles

                def emit_qkv(nb):
                    xt = pB.tile([128, 6, 128], BF16, tag="xt")
                    nc.sync.dma_start(
                        xt[:],
                        x_t.rearrange("(ck p) n -> p ck n", p=128)[
                            :, :, nb * 128 : (nb + 1) * 128
                        ],
                    )
                    if nb == 0:
                        # remaining weight slices land while the first x tile
                        # is being consumed
                        for ck in range(1, 6):
                            nc.sync.dma_start(wq[:, ck, :], wq_src[:, ck, :])
                        nc.sync.dma_start(gbct[:], gbc[:, :])
                    # q|k at cols 0:384 (bank 0), v at 512:704 (bank 1)
                    ps = psQ.tile([128, 1024], F32, tag="qkvps")
                    for off, woff, w in ((0, 0, 384), (512, 384, 192)):
                        for ck in range(6):
                            nc.tensor.matmul(
                                ps[:, off : off + w],
                                xt[:, ck, :],
                                wq[:, ck, woff : woff + w],
                                start=(ck == 0),
                                stop=(ck == 5),
                            )
                    st[nb] = {"ps": ps}

                def emit_stats(nb):
                    t = st[nb]
                    ps = t.pop("ps")
                    qkvS = pBs.tile([128, G, D], BF16, tag="qkvS")
                    nc.scalar.copy(
                        qkvS[:], ps[:, 0 : G * D].rearrange("p (g d) -> p g d", d=D)
                    )
                    nc.scalar.copy(
                        vA[:, :, nb, 0:64],
                        ps[:, 512 : 512 + H * D].rearrange("p (h d) -> p h d", d=D),
                    )
                    s1 = pBs.tile([128, G], F32, tag="s1")
                    nc.vector.tensor_reduce(s1[:], qkvS[:], AXL.X, ALU.add)
                    mu = pBs.tile([128, G], F32, tag="mu")
                    nc.vector.tensor_scalar_mul(mu[:], s1[:], 1.0 / D)
                    sq = pBs.tile([128, G, D], BF16, tag="sq")
                    nc.vector.tensor_mul(sq[:], qkvS[:], qkvS[:])
                    s2 = pBs.tile([128, G], F32, tag="s2")
                    nc.vector.tensor_reduce(s2[:], sq[:], AXL.X, ALU.add)
                    musq = pBs.tile([128, G], F32, tag="musq")
                    nc.vector.tensor_mul(musq[:], mu[:], mu[:])
                    var = pBs.tile([128, G], F32, tag="var")
                    nc.vector.scalar_tensor_tensor(
                        var[:], s2[:], 1.0 / D, musq[:], ALU.mult, ALU.subtract
                    )
                    std = pBs.tile([128, G], F32, tag="std")
                    nc.scalar.activation(std[:], var[:], ACTF.Sqrt, bias=epst[:])
                    rstd = pBs.tile([128, G], F32, tag="rstd")
                    nc.vector.reciprocal_approx_fast(rstd[:], std[:])
                    t.update(qkvS=qkvS, mu=mu, rstd=rstd)

                def emit_apply(nb):
                    t = st[nb]
                    qkvS, mu, rstd = t.pop("qkvS"), t.pop("mu"), t.pop("rstd")
                    # cs = (x - mu) * rstd on gpsimd; gamma/beta are applied
                    # later, inside the post-transpose per-partition copies
                    cst = pBs.tile([128, G, D], BF16, tag="cst")
                    nc.gpsimd.tensor_sub(
                        cst[:], qkvS[:], mu[:, :, None].broadcast_to([128, G, D])
                    )
                    cs = pBs.tile([128, G, D], BF16, tag="cs")
                    nc.gpsimd.tensor_mul(
                        cs[:], cst[:], rstd[:, :, None].broadcast_to([128, G, D])
                    )
                    t["cs"] = cs

                def emit_transp(nb):
                    t = st[nb]
                    cs = t.pop("cs")
                    pst = psT.tile([128, G, 128], BF16, tag="pst")
                    jh = nb // (NB // 2)
                    for g in range(H):
                        # q slabs duplicated over both 64-row halves via two
                        # transposes of the same stationary
                        for half in range(2):
                            nc.tensor.transpose(
                                pst[64 * half : 64 * half + 64, g, :],
                                cs[:, g, :],
                                identb[:],
                                tile_position=(0, 64 * half),
                            )
                    for g in range(H, G):
                        # k slabs only need the jh half
                        nc.tensor.transpose(
                            pst[64 * jh : 64 * jh + 64, g, :],
                            cs[:, g, :],
                            identb[:],
                            tile_position=(0, 64 * jh),
                        )
                    blk = slice(nb * 128, (nb + 1) * 128)
                    # copies apply gamma/beta per partition (= per d after
                    # the transpose): out = in * gamma_col + beta_col
                    nc.scalar.activation(
                        qT[:, :, blk], pst[:, 0:H, :], ACTF.Identity,
                        bias=gbct[:, 1:2], scale=gbct[:, 0:1],
                    )
                    cb = nb % (NB // 2)
                    rows = slice(64 * jh, 64 * jh + 64)
                    nc.scalar.activation(
                        kT[rows, :, cb * 128 : (cb + 1) * 128], pst[rows, H:G, :],
                        ACTF.Identity,
                        bias=gbct[rows, 3:4], scale=gbct[rows, 2:3],
                    )
                    del st[nb]

                for s in range(NB + 3):
                    if s == 3:
                        nc.sync.dma_start(pw128[:], projw_t[0:128, :].bitcast(F32R))
                        nc.sync.dma_start(pw64[:], projw_t[128:192, :].bitcast(F32R))
                    if s >= 2 and s - 2 < NB:
                        emit_apply(s - 2)
                    if s >= 3:
                        emit_transp(s - 3)
                    if s < NB:
                        emit_qkv(s)
                    if s >= 1 and s - 1 < NB:
                        emit_stats(s - 1)

            # ================= Phase C: attention + proj =================
            NT = (NB + 2) // 3          # score/exp tiles per (ib, h)
            with (
                tc.tile_pool(name="pt", bufs=8) as ptp,
                tc.tile_pool(name="pCs", bufs=6) as pCs,
                tc.tile_pool(name="pD", bufs=3) as pD,
                tc.tile_pool(name="psS", bufs=2, space="PSUM") as psS,
                tc.tile_pool(name="psOD", bufs=2, space="PSUM") as psOD,
            ):
                # chunk order alternates PE quadrants so adjacent score
                # matmuls run concurrently: 0,16,1,17,2,18,...
                corder = []
                for i in range(NB // 2):
                    corder.append(i)
                    corder.append(i + NB // 2)

                pvq = []      # pending PV closures, global FIFO across heads
                side = []     # pending (ib, proj-unit) from the previous i-block
                ao_done = set()   # i-blocks whose h2 normalize has been emitted

                def pop_pv():
                    pvq.pop(0)()

                def pop_side():
                    if side and side[0][0] in ao_done:
                        side.pop(0)[1]()

                def normalize(ib, h, pso):
                    isl = slice(ib * 512, (ib + 1) * 512)
                    rden_f = pCs.tile([1, 512], F32, tag="rden_f")
                    nc.vector.tensor_copy(rden_f[:], pso[64:65, :])
                    rden = pCs.tile([1, 512], F32, tag="rden")
                    nc.vector.reciprocal_approx_fast(rden[:], rden_f[:])
                    rb = pCs.tile([64, 512], F32, tag="rb")
                    nc.gpsimd.partition_broadcast(rb[:], rden[:])
                    if h == 0:
                        nc.vector.tensor_mul(ao1[0:64, isl], pso[0:64, :], rb[:])
                    elif h == 2:
                        nc.vector.tensor_mul(ao2[0:64, isl], pso[0:64, :], rb[:])
                    else:
                        stg = pCs.tile([64, 512], F32R, tag="stg")
                        nc.vector.tensor_mul(stg[:], pso[0:64, :], rb[:])
                        nc.sync.dma_start(ao1[64:128, isl], stg[:])
                    if h == 2:
                        ao_done.add(ib)

                def make_pv(pso, h, jc, idx, ib):
                    def run():
                        nc.tensor.matmul(
                            pso,
                            vA[:, h, jc, :],
                            pvq_pt.pop((ib, h, idx)),
                            start=(idx == 0),
                            stop=(idx == NB - 1),
                        )
                        if idx == NB - 1:
                            normalize(ib, h, pso)
                    return run

                pvq_pt = {}

                def make_proj(ib):
                    units = []
                    for nb in range(ib * 4, ib * 4 + 4):
                        blk = slice(nb * 128, (nb + 1) * 128)
                        stage = [None]

                        def u1(blk=blk, stage=stage):
                            stage[0] = pD.tile([128, C], F32, tag="stage", name="stage")
                            pd_t = psOD.tile([128, 512], F32, tag="psod", name="pd")
                            nc.tensor.matmul(
                                pd_t[:, 0:512], r32(ao1[:, blk]),
                                r32(pw128[:, 0:512]), start=True, stop=False,
                            )
                            nc.tensor.matmul(
                                pd_t[:, 0:512], r32(ao2[0:64, blk]),
                                r32(pw64[0:64, 0:512]), start=False, stop=True,
                            )
                            nc.vector.tensor_copy(stage[0][:, 0:512], pd_t[:, 0:512])

                        def u2(blk=blk, stage=stage):
                            pd_t = psOD.tile([128, 512], F32, tag="psod", name="pd")
                            nc.tensor.matmul(
                                pd_t[:, 0:256], r32(ao1[:, blk]),
                                r32(pw128[:, 512:768]), start=True, stop=False,
                            )
                            nc.tensor.matmul(
                                pd_t[:, 0:256], r32(ao2[0:64, blk]),
                                r32(pw64[0:64, 512:768]), start=False, stop=True,
                            )
                            nc.vector.tensor_copy(stage[0][:, 512:768], pd_t[:, 0:256])
                            nc.sync.dma_start(out_p[blk, :], stage[0][:])

                        units.append((ib, u1))
                        units.append((ib, u2))
                    return units

                for ib in range(IB):
                    isl = slice(ib * 512, (ib + 1) * 512)
                    for h in range(H):
                        pso_t = psOD.tile([128, 512], F32, tag="psod", name="pso")
                        pso = pso_t[0:65, :]
                        for t in range(NT):
                            nch = min(3, NB - 3 * t)
                            W = 512 * nch
                            ps = psS.tile([128, 1536], F32, tag="st")
                            for s in range(nch):
                                jc = corder[3 * t + s]
                                p0 = 0 if jc < NB // 2 else 64
                                jf = (jc % (NB // 2)) * 128
                                nc.tensor.matmul(
                                    ps[:, 512 * s : 512 * s + 512],
                                    kT[p0 : p0 + 64, h, jf : jf + 128],
                                    qT[p0 : p0 + 64, h, isl],
                                    start=True,
                                    stop=True,
                                    tile_position=(p0, 0),
                                )
                                # keep PV/proj matmuls out of the middle of a
                                # score pair (adjacent scores alternate PE
                                # quadrants and execute concurrently), and
                                # never pop during a head's first tile so the
                                # exp stream restarts without delay; bounded
                                # pops avoid bursts that would delay s2
                                if t == 0:
                                    pass
                                elif s == 0 and len(pvq) > SKEW_CHUNKS + 2:
                                    pop_pv()
                                elif s == 1:
                                    if len(pvq) > SKEW_CHUNKS:
                                        pop_pv()
                                    if len(pvq) > SKEW_CHUNKS + 1:
                                        pop_pv()
                                    if t >= 4:
                                        pop_side()
                                elif s == 2 and len(pvq) > SKEW_CHUNKS:
                                    pop_pv()
                            pt = ptp.tile([128, 1536], BF16, tag="pt")
                            nc.scalar.activation(pt[:, 0:W], ps[:, 0:W], ACTF.Exp)
                            for s in range(nch):
                                idx = 3 * t + s
                                pvq_pt[(ib, h, idx)] = pt[:, 512 * s : 512 * s + 512]
                                pvq.append(make_pv(pso, h, corder[idx], idx, ib))
                    # queue this i-block's projection for the next i-block
                    side.extend(make_proj(ib))
                    if ib == IB - 1:
                        while pvq:
                            pop_pv()
                        while side:
                            side.pop(0)[1]()

    nc.compile()
    return nc


@lru_cache(maxsize=2)
def _built(N):
    nc = build_nc(N)
    return nc


def _prep_inputs(x, qkv_w, q_gamma, q_beta, k_gamma, k_beta, proj_w):
    x = np.asarray(x, np.float32)
    qkv_w = np.asarray(qkv_w, np.float32)
    proj_w = np.asarray(proj_w, np.float32)
    B = x.shape[0]
    import ml_dtypes
    xts = [np.ascontiguousarray(x[b].T).astype(ml_dtypes.bfloat16) for b in range(B)]
    gb2 = np.stack(
        [
            np.tile(np.asarray(q_gamma, np.float32) * SCALE, 2),
            np.tile(np.asarray(q_beta, np.float32) * SCALE, 2),
            np.tile(np.asarray(k_gamma, np.float32), 2),
            np.tile(np.asarray(k_beta, np.float32), 2),
        ],
        axis=1,
    )  # [128, 4]
    gbs = []
    wqs = []
    pws = []
    for g in range(4):
        r = slice(192 * g, 192 * (g + 1))
        wq_rows = np.concatenate(
            [qkv_w[r], qkv_w[768:1536][r], qkv_w[1536:2304][r]], axis=0
        )
        wqs.append(np.ascontiguousarray(wq_rows.T).astype(ml_dtypes.bfloat16))
        pws.append(np.ascontiguousarray(proj_w[:, r].T))
        gbs.append(gb2)
    in_maps = []
    for core in range(8):
        b, g = core // 4, core % 4
        in_maps.append(
            {"x_t": xts[b], "wqkv_t": wqs[g], "projw_t": pws[g], "gbc": gbs[g]}
        )
    return in_maps


def run_cores(in_maps, N, trace=False):
    from concourse.bass_utils import run_bass_kernel_spmd

    nc = _built(N)
    res = run_bass_kernel_spmd(nc, in_maps, list(range(8)), trace=trace)
    return res


def kernel(x, qkv_w, q_gamma, q_beta, k_gamma, k_beta, proj_w, proj_b):
    x = np.asarray(x, np.float32)
    N = x.shape[1]
    in_maps = _prep_inputs(x, qkv_w, q_gamma, q_beta, k_gamma, k_beta, proj_w)
    res = run_cores(in_maps, N)
    parts = [np.asarray(r["out_p"], np.float32) for r in res.results]
    out0 = parts[0] + parts[1] + parts[2] + parts[3]
    out1 = parts[4] + parts[5] + parts[6] + parts[7]
    out = np.stack([out0, out1]) + np.asarray(proj_b, np.float32)
    return out.astype(np.float32)
